# revision 16
# baseline (speedup 1.0000x reference)
import sys

if "/opt/trn_rl_repo" not in sys.path:
    sys.path.insert(0, "/opt/trn_rl_repo")

import numpy as np
from contextlib import ExitStack

from concourse import bass, bacc, mybir, tile
from concourse.bass_utils import run_bass_kernel_spmd

B, O, I, CI, CO = 64, 32, 1024, 16, 16
NCORES = 8
IL = I // NCORES  # 128 i's per core
OD = O * CO       # 512

f32 = mybir.dt.float32
f32r = mybir.dt.float32r
AF = mybir.ActivationFunctionType
OP = mybir.AluOpType
AX = mybir.AxisListType


def _build():
    nc = bacc.Bacc(None, target_bir_lowering=False, debug=True)

    dataT_d = nc.declare_dram_parameter("dataT", [32, 128, 64], f32r, isOutput=False)
    Wt_d = nc.declare_dram_parameter("Wt", [32, 128, OD], f32r, isOutput=False)
    bias_d = nc.declare_dram_parameter("bias_od", [64, OD], f32, isOutput=False)
    alpha_d = nc.declare_dram_parameter("alpha_bo", [64, O], f32, isOutput=False)
    beta_d = nc.declare_dram_parameter("beta_bo", [64, O], f32, isOutput=False)
    out_d = nc.declare_dram_parameter("out", [64, OD], f32, isOutput=True)

    with tile.TileContext(nc) as tc, ExitStack() as ctx:
        def pool(name, **kw):
            return ctx.enter_context(tc.tile_pool(name=name, **kw))

        def t1(name, shape, dtype=f32):
            return pool(name, bufs=1).tile(shape, dtype, name=name)

        # persistent SBUF tiles
        UH = t1("UH", [128, O * 64 * CO])      # u_hat: p=(i0,b), col = o*1024 + i2*16 + d
        s_red = t1("s_red", [128, OD])
        s_hi = t1("s_hi", [64, OD])
        s_stage = t1("s_stage", [64, OD])
        s_full = t1("s_full", [64, OD])
        sB = t1("sB", [64, OD])
        v = t1("v", [64, OD])
        v2 = t1("v2", [128, OD])
        t512 = t1("t512", [64, OD])
        outsb = t1("outsb", [64, OD])
        biassb = t1("biassb", [64, OD])
        alphasb = t1("alphasb", [64, O])
        betasb = t1("betasb", [64, O])
        Z = t1("Z", [128, 64])
        Zc = t1("Zc", [128, 64])
        sq = t1("sq", [64, O])
        w1 = t1("w1", [64, O])
        r1 = t1("r1", [64, O])
        u1 = t1("u1", [64, O])
        l1 = t1("l1", [64, O])
        rs = t1("rs", [64, O])
        g = t1("g", [64, O])
        z1 = t1("z1", [64, O])
        eg = t1("eg", [64, O])
        ag = t1("ag", [64, O])
        eps = t1("eps", [128, 1])

        pe = pool("pe", bufs=4, space=bass.MemorySpace.PSUM)
        s1 = pool("s1", bufs=1, space=bass.MemorySpace.PSUM).tile([64, OD], f32)
        dram = pool("dram", bufs=6, space="DRAM")

        UH4 = UH[:].rearrange("p (o i d) -> p o i d", o=O, i=64)

        # ---- input DMAs ----
        nc.sync.dma_start(biassb[:], bias_d[:])
        nc.sync.dma_start(alphasb[:], alpha_d[:])
        nc.sync.dma_start(betasb[:], beta_d[:])
        nc.gpsimd.memset(eps[:], 1e-8)

        # ---- phase B: u_hat + s1 partial via PE ----
        with tc.tile_pool(name="dsbp", bufs=1) as dsbp, \
             tc.tile_pool(name="w", bufs=2) as wpool:
            dsb = dsbp.tile([128, 32 * 64], f32r, name="dsb")
            nc.sync.dma_start(
                dsb[:].rearrange("p (t b) -> p t b", t=32),
                dataT_d[:].rearrange("t p b -> p t b"),
            )
            for b2 in range(16):
                wA = wpool.tile([128, OD], f32r)
                nc.sync.dma_start(wA[:], Wt_d[b2])
                wB = wpool.tile([128, OD], f32r)
                nc.sync.dma_start(wB[:], Wt_d[16 + b2])

                # s1 += sum_{i in tiles b2, 16+b2} sum_c data*W  (K=128, pad rows are 0)
                nc.tensor.matmul(
                    s1[:, :],
                    dsb[:, 64 * b2:64 * b2 + 64],
                    wA[:],
                    start=(b2 == 0), stop=False, skip_group_check=True,
                )
                nc.tensor.matmul(
                    s1[:, :],
                    dsb[:, 64 * (16 + b2):64 * (16 + b2) + 64],
                    wB[:],
                    start=False, stop=(b2 == 15), skip_group_check=True,
                )

                for j in range(4):
                    i2 = 4 * b2 + j
                    ptA = pe.tile([64, OD], f32, name="pt")
                    ptB = pe.tile([64, OD], f32, name="pt")
                    nc.tensor.matmul(
                        ptA[:, :],
                        dsb[32 * j:32 * j + 16, 64 * b2:64 * b2 + 64],
                        wA[32 * j:32 * j + 16, :],
                        start=True, stop=True, tile_position=(32 * j, 0),
                    )
                    nc.tensor.matmul(
                        ptB[:, :],
                        dsb[32 * j:32 * j + 16, 64 * (16 + b2):64 * (16 + b2) + 64],
                        wB[32 * j:32 * j + 16, :],
                        start=True, stop=True, tile_position=(32 * j, 0),
                    )
                    nc.vector.tensor_copy(
                        UH4[0:64, :, i2, :],
                        ptA[:].rearrange("p (o d) -> p o d", d=CO))
                    nc.scalar.copy(
                        UH4[64:128, :, i2, :],
                        ptB[:].rearrange("p (o d) -> p o d", d=CO))

        # phase-C workspace pools (allocated after phase B frees dsb/w)
        bl = t1("bl", [128, O * 64])           # b_log: col = o*64 + i2
        E = t1("E", [128, O * 64])             # exp / coupling coeffs
        x = t1("x", [128, 4 * 64 * CO])        # chunk workspace (4 o's)
        at_ = t1("at", [128, 4 * 64])          # agreement chunk
        BL3 = bl[:].rearrange("p (o i) -> p o i", o=O)
        E3 = E[:].rearrange("p (o i) -> p o i", o=O)
        X4 = x[:].rearrange("p (o i d) -> p o i d", o=4, i=64)

        # ---- AllReduce helper ----
        def allreduce(src_t, dst_t):
            bi = dram.tile([64, OD], f32)
            bo = dram.tile([64, OD], f32)
            nc.gpsimd.dma_start(bi[:], src_t[:])
            nc.gpsimd.collective_compute(
                "AllReduce", OP.add,
                replica_groups=[list(range(NCORES))],
                ins=[bi.opt()], outs=[bo.opt()],
            )
            nc.gpsimd.dma_start(dst_t[:], bo[:])

        def squash(s_in, v_out):
            nc.scalar.square(t512[:], s_in[:])
            nc.vector.tensor_reduce(
                sq[:], t512[:].rearrange("p (o d) -> p o d", d=CO), AX.X, OP.add)
            nc.vector.tensor_scalar_add(w1[:], sq[:], 1.0)
            nc.vector.reciprocal(r1[:], w1[:])
            nc.vector.tensor_tensor(u1[:], sq[:], r1[:], OP.mult)
            nc.scalar.activation(l1[:], sq[:], AF.Ln, bias=eps[0:64, :], scale=1.0)
            nc.scalar.activation(rs[:], l1[:], AF.Exp, bias=0.0, scale=-0.5)
            nc.vector.tensor_tensor(g[:], u1[:], rs[:], OP.mult)
            nc.vector.tensor_tensor(
                v_out[:].rearrange("p (o d) -> p o d", d=CO),
                s_in[:].rearrange("p (o d) -> p o d", d=CO),
                g[:].unsqueeze(2).broadcast_to([64, O, CO]),
                OP.mult)

        # ---- phase C: routing iterations ----
        for t in range(3):
            if t == 0:
                nc.scalar.mul(s_stage[:], s1[:], 1.0 / O)
            else:
                nc.scalar.activation(E[:], bl[:], AF.Exp)
                nc.vector.tensor_reduce(
                    Z[:], E[:].rearrange("p (o i) -> p i o", o=O), AX.X, OP.add)
                nc.vector.reciprocal(Zc[:], Z[:])
                nc.vector.tensor_tensor(
                    E3, E3, Zc[:].unsqueeze(1).broadcast_to([128, O, 64]), OP.mult)
                for k in range(8):
                    nc.vector.tensor_tensor(
                        X4, UH4[:, 4 * k:4 * k + 4, :, :],
                        E3[:, 4 * k:4 * k + 4, :].unsqueeze(3).broadcast_to([128, 4, 64, CO]),
                        OP.mult)
                    nc.vector.tensor_reduce(
                        s_red[:, 64 * k:64 * k + 64].rearrange("p (o d) -> p o d", d=CO),
                        X4.transpose([0, 1, 3, 2]), AX.X, OP.add)
                nc.scalar.copy(s_hi[:], s_red[64:128, :])
                nc.vector.tensor_tensor(
                    s_stage[:], s_red[0:64, :], s_hi[:], OP.add)

            allreduce(s_stage, s_full)
            nc.vector.tensor_tensor(sB[:], s_full[:], biassb[:], OP.add)
            squash(sB, v)

            if t < 2:
                nc.gpsimd.dma_start(v2[0:64, :], v[:])
                nc.gpsimd.dma_start(v2[64:128, :], v[:])
                for k in range(8):
                    nc.vector.tensor_tensor(
                        X4, UH4[:, 4 * k:4 * k + 4, :, :],
                        v2[:].rearrange("p (o d) -> p o d", d=CO)[:, 4 * k:4 * k + 4, :]
                            .unsqueeze(2).broadcast_to([128, 4, 64, CO]),
                        OP.mult)
                    if t == 0:
                        nc.vector.tensor_reduce(
                            BL3[:, 4 * k:4 * k + 4, :], X4, AX.X, OP.add)
                    else:
                        nc.vector.tensor_reduce(
                            at_[:].rearrange("p (o i) -> p o i", o=4), X4, AX.X, OP.add)
                        blk = bl[:, 256 * k:256 * k + 256]
                        nc.vector.tensor_tensor(blk, blk, at_[:], OP.add)

        # ---- final activation gate ----
        nc.scalar.square(t512[:], v[:])
        nc.vector.tensor_reduce(
            sq[:], t512[:].rearrange("p (o d) -> p o d", d=CO), AX.X, OP.add)
        nc.scalar.activation(l1[:], sq[:], AF.Ln, bias=eps[0:64, :], scale=1.0)
        nc.scalar.activation(z1[:], l1[:], AF.Exp, bias=0.0, scale=0.5)  # norm
        nc.vector.tensor_tensor(z1[:], z1[:], alphasb[:], OP.mult)
        nc.vector.tensor_tensor(z1[:], z1[:], betasb[:], OP.add)
        nc.scalar.activation(eg[:], z1[:], AF.Exp, bias=0.0, scale=-1.0)
        nc.vector.tensor_scalar_add(eg[:], eg[:], 1.0)
        nc.vector.reciprocal(ag[:], eg[:])
        nc.vector.tensor_tensor(
            outsb[:].rearrange("p (o d) -> p o d", d=CO),
            v[:].rearrange("p (o d) -> p o d", d=CO),
            ag[:].unsqueeze(2).broadcast_to([64, O, CO]),
            OP.mult)
        nc.sync.dma_start(out_d[:], outsb[:])

    nc.compile()
    return nc


def _prep_maps(data, W, bias, alpha, beta):
    data = np.ascontiguousarray(data, dtype=np.float32)
    W = np.ascontiguousarray(W, dtype=np.float32)
    bias_od = np.repeat(bias.astype(np.float32), CO)[None, :].repeat(64, axis=0).copy()
    alpha_bo = alpha.astype(np.float32)[None, :].repeat(64, axis=0).copy()
    beta_bo = beta.astype(np.float32)[None, :].repeat(64, axis=0).copy()
    maps = []
    for k in range(NCORES):
        dc = data[:, IL * k:IL * (k + 1), :]          # [64,128,16]
        wc = W[:, IL * k:IL * (k + 1), :, :]          # [32,128,16,16]
        dT = dc.transpose(1, 2, 0)                    # [i,c,b]
        dpad = np.zeros((2, 16, 4, 32, 64), np.float32)
        dpad[:, :, :, :16, :] = dT.reshape(2, 16, 4, 16, 64)
        wT = wc.transpose(1, 2, 0, 3).reshape(128, CI, OD)   # [i,c,(o,d)]
        wpad = np.zeros((2, 16, 4, 32, OD), np.float32)
        wpad[:, :, :, :16, :] = wT.reshape(2, 16, 4, 16, OD)
        maps.append(dict(
            dataT=np.ascontiguousarray(dpad.reshape(32, 128, 64)),
            Wt=np.ascontiguousarray(wpad.reshape(32, 128, OD)),
            bias_od=bias_od, alpha_bo=alpha_bo, beta_bo=beta_bo,
        ))
    return maps


_NC_CACHE = None


def kernel(data, W, bias, beta, alpha, size):
    global _NC_CACHE
    if _NC_CACHE is None:
        _NC_CACHE = _build()
    maps = _prep_maps(np.asarray(data), np.asarray(W), np.asarray(bias),
                      np.asarray(alpha), np.asarray(beta))
    res = run_bass_kernel_spmd(_NC_CACHE, maps, list(range(NCORES)))
    out = np.asarray(res.results[0]["out"], dtype=np.float32)
    return out.reshape(B, O, CO)
